# revision 2
# baseline (speedup 1.0000x reference)
"""CDGRL (gnn_message_passing) Trainium2 kernel — 8-core SPMD, v2.

Uniform (no partition-id branches) restructure of the baseline:
- Interleaved row ownership: core c owns x1[256c:256c+256] ++ x2[256c:256c+256],
  so every core's 512 rows are half domain-0 / half domain-1 and the program
  is identical on all cores.
- S phase: feature-sharded Gram. AllToAll distributes fp8(xn*64) feature
  slices; each core computes the full inter-domain S12/S21 partial for its
  512-feature slice; one bf16 ReduceScatter delivers each core its own
  S row-block [own 512 x opposite-domain 2048]. Replaces the 33.5MB xnT
  AllGather.
- GCN layers: partial A@Y products from own columns + ReduceScatter,
  with D^-1/2 folded into local pre/post scalings (Y' = dinv*Y before the
  matmul, dinv_r applied post-RS on the owning core) so no degree exchange
  is needed at all. Self-loop diagonal added locally post-RS.
- Tiny AllGathers replace the cm/wv/deg/loss AllReduces.
Collective out-bytes drop ~8x vs baseline; no tc.If so TimelineSim works.
"""

import numpy as np
import ml_dtypes

N = 4096
D = 4096
K = 21
NC = 8
R = 512           # own rows per core (2 halves of 256: d0 then d1)
RT = 4            # 128-row tiles per core (t 0,1 = d0; t 2,3 = d1)
DT = 32           # 128-chunks of D
EPS = 1e-8
F1 = 2048
F2 = 1024
F3 = 512
F4 = 256
QS = 64.0         # fp8 quant scale for normalized rows
GQ = 8            # (dom,k) pairs per core in the q ReduceScatter shard


def _build():
    import concourse.bass as bass
    import concourse.mybir as mybir
    import concourse.tile as tile
    from concourse import bacc
    from concourse.masks import make_identity

    dt = mybir.dt
    AX = mybir.AxisListType.X
    OP = mybir.AluOpType
    ACT = mybir.ActivationFunctionType

    nc = bacc.Bacc("TRN2", target_bir_lowering=False, debug=False, num_devices=NC)

    xb = nc.dram_tensor("xb", [R, D], dt.float32, kind="ExternalInput")
    ohc_d = nc.dram_tensor("ohc", [R, K], dt.float32, kind="ExternalInput")
    ohdiv_d = nc.dram_tensor("ohdiv", [R, K], dt.float32, kind="ExternalInput")
    ohall_d = nc.dram_tensor("ohall", [N, K], dt.float32, kind="ExternalInput")
    w1_d = nc.dram_tensor("w1b", [D, F1], dt.bfloat16, kind="ExternalInput")
    w2_d = nc.dram_tensor("w2b", [F1, F2], dt.bfloat16, kind="ExternalInput")
    fw1_d = nc.dram_tensor("fw1b", [F2, F3], dt.bfloat16, kind="ExternalInput")
    fw2_d = nc.dram_tensor("fw2b", [F3, F4], dt.bfloat16, kind="ExternalInput")
    fw3_d = nc.dram_tensor("fw3b", [F4, K], dt.bfloat16, kind="ExternalInput")
    b1t_d = nc.dram_tensor("b1t", [128, F1 // 128], dt.float32, kind="ExternalInput")
    b2t_d = nc.dram_tensor("b2t", [128, F2 // 128], dt.float32, kind="ExternalInput")
    fb1t_d = nc.dram_tensor("fb1t", [128, F3 // 128], dt.float32, kind="ExternalInput")
    fb2t_d = nc.dram_tensor("fb2t", [128, F4 // 128], dt.float32, kind="ExternalInput")
    fb3_d = nc.dram_tensor("fb3c", [K, 1], dt.float32, kind="ExternalInput")
    loss_d = nc.dram_tensor("loss", [1, 1], dt.float32, kind="ExternalOutput")

    with tile.TileContext(nc) as tc:
        with (
            tc.tile_pool(name="dram", bufs=1, space="DRAM") as dram,
            tc.tile_pool(name="pers", bufs=1) as pers,
            tc.tile_pool(name="pp_g", bufs=3, space="PSUM") as pp_g,
            tc.tile_pool(name="pp_t", bufs=2, space="PSUM") as pp_t,
            tc.tile_pool(name="pp_sm", bufs=2, space="PSUM") as pp_sm,
        ):
            # ---- collective DRAM buffers ----
            q_in = dram.tile([2, DT, 128, K], dt.float32)
            q_rs = dram.tile([GQ, 128, K], dt.float32)
            ctq_in = dram.tile([GQ, 128, K], dt.bfloat16)
            ct_all = dram.tile([2 * DT, 128, K], dt.bfloat16)
            a2a_in = dram.tile([NC, 4, 128, R], dt.float8e4)
            a2a_out = dram.tile([NC, 4, 128, R], dt.float8e4)
            wv_in = dram.tile([R], dt.float32)
            wv_all = dram.tile([N], dt.float32, addr_space="Shared")
            rsg_in = [dram.tile([N, 1024], dt.bfloat16, name=f"rsgin{i}") for i in range(2)]
            rsg_out = [dram.tile([R, 1024], dt.bfloat16, name=f"rsgout{i}") for i in range(2)]
            rs1_in = [dram.tile([N, F1 // 2], dt.bfloat16, name=f"rs1in{i}") for i in range(2)]
            rs1_out = [dram.tile([R, F1 // 2], dt.bfloat16, name=f"rs1out{i}") for i in range(2)]
            rs2_in = dram.tile([N, F2], dt.bfloat16)
            rs2_out = dram.tile([R, F2], dt.bfloat16)
            ls_in = dram.tile([1], dt.float32)
            ls_out = dram.tile([NC], dt.float32, addr_space="Shared")

            GRP_ALL = [list(range(NC))]

            def cc(kind, op, i, o):
                nc.gpsimd.collective_compute(
                    kind, op, replica_groups=GRP_ALL, ins=[i.opt()], outs=[o.opt()]
                )

            # ---- persistent SBUF ----
            eye_f = pers.tile([128, 128], dt.float32)
            make_identity(nc, eye_f[:])
            ones_c = pers.tile([128, 1], dt.float32)
            nc.vector.memset(ones_c[:], 1.0)
            ohc = pers.tile([128, RT, K], dt.float32)
            nc.sync.dma_start(ohc[:], ohc_d.rearrange("(t p) k -> p t k", p=128))
            ohdiv = pers.tile([128, RT, K], dt.float32)
            nc.sync.dma_start(ohdiv[:], ohdiv_d.rearrange("(t p) k -> p t k", p=128))
            b1t = pers.tile([128, F1 // 128], dt.float32)
            nc.sync.dma_start(b1t[:], b1t_d[:])
            b2t = pers.tile([128, F2 // 128], dt.float32)
            nc.sync.dma_start(b2t[:], b2t_d[:])
            fb1t = pers.tile([128, F3 // 128], dt.float32)
            nc.sync.dma_start(fb1t[:], fb1t_d[:])
            fb2t = pers.tile([128, F4 // 128], dt.float32)
            nc.sync.dma_start(fb2t[:], fb2t_d[:])
            fb3 = pers.tile([K, 1], dt.float32)
            nc.sync.dma_start(fb3[:], fb3_d[:])

            xnT = pers.tile([128, DT, R], dt.bfloat16)
            XW1 = pers.tile([128, RT, F1], dt.bfloat16)   # becomes Y1' in place
            A_blk = pers.tile([128, RT, 2048], dt.bfloat16)
            h1T = pers.tile([128, F1 // 128, R], dt.bfloat16)
            H2f32 = pers.tile([128, RT, F2], dt.float32)
            H2 = pers.tile([128, RT, F2], dt.bfloat16)    # Y2' = dinv*H2
            h2T = pers.tile([128, F2 // 128, R], dt.bfloat16)
            cnb = pers.tile([128, 2, K], dt.float32)
            cmxb = pers.tile([128, 2, K], dt.float32)
            wjb0 = pers.tile([128, 2048], dt.float32)     # w over d0 cols
            wjb1 = pers.tile([128, 2048], dt.float32)     # w over d1 cols
            w_own = pers.tile([128, RT], dt.float32)
            simi = pers.tile([128, RT], dt.float32)
            simi_all = pers.tile([128, N // 128], dt.float32)
            norm_r = pers.tile([128, RT], dt.float32)
            norm_b = pers.tile([128, RT], dt.float32)
            norm_c = pers.tile([128, RT], dt.float32)
            norm_d = pers.tile([128, RT], dt.float32)
            ninv_r = pers.tile([128, RT], dt.float32)
            ninvj = pers.tile([1, R], dt.float32)
            ninvjb = pers.tile([128, R], dt.float32)
            dinv = pers.tile([128, RT], dt.float32)
            deg = pers.tile([128, RT], dt.float32)

            normq = [norm_r, norm_b, norm_c, norm_d]

            # ============ P0: x load, norms, xn transpose, fp8 ship, q ======
            with tc.tile_pool(name="p0", bufs=1) as p0:
                xrow = p0.tile([128, RT, D], dt.float32)
                xbv = xb.rearrange("(t p) d -> p t d", p=128)
                for t in range(RT):
                    nc.sync.dma_start(xrow[:, t, :], xbv[:, t, :])

                # row norms via ACT Square accumulate (4 quarter-D passes/tile)
                for t in range(RT):
                    for hh in range(4):
                        sq = p0.tile([128, D // 4], dt.float32, tag="sq", bufs=1,
                                     name=f"sq{t}_{hh}")
                        nc.scalar.activation(
                            sq[:], xrow[:, t, (D // 4) * hh : (D // 4) * (hh + 1)],
                            ACT.Square,
                            accum_out=normq[hh][:, t : t + 1],
                        )
                for hh in range(1, 4):
                    nc.vector.tensor_tensor(norm_r[:], norm_r[:], normq[hh][:],
                                            OP.add)
                nc.scalar.activation(norm_r[:], norm_r[:], ACT.Sqrt)
                nc.vector.tensor_scalar(ninv_r[:], norm_r[:], EPS, None, OP.max)
                nc.vector.reciprocal(ninv_r[:], ninv_r[:])
                for t in range(RT):
                    pw = pp_sm.tile([1, 128], dt.float32, tag="sm", name=f"nv{t}")
                    nc.tensor.transpose(pw[:], ninv_r[:, t : t + 1], eye_f[:])
                    nc.vector.tensor_copy(ninvj[:, 128 * t : 128 * (t + 1)], pw[:])
                nc.gpsimd.partition_broadcast(ninvjb[:], ninvj[:])

                # transpose raw x (batched 4-wide into one PSUM bank);
                # normalize during evacuation with repeated-ninv rows
                ninv4 = p0.tile([128, RT, 512], dt.float32)
                for t in range(RT):
                    for i in range(4):
                        nc.vector.tensor_copy(
                            ninv4[:, t, 128 * i : 128 * (i + 1)],
                            ninvjb[:, 128 * t : 128 * (t + 1)],
                        )
                for t in range(RT):
                    for kg in range(8):
                        ps = pp_t.tile([128, 512], dt.float32, tag="tp",
                                       name=f"tp{kg}_{t}")
                        for i in range(4):
                            nc.tensor.transpose(
                                ps[:, 128 * i : 128 * (i + 1)],
                                xrow[:, t, 512 * kg + 128 * i :
                                     512 * kg + 128 * (i + 1)],
                                eye_f[:],
                            )
                        nc.vector.tensor_tensor(
                            xnT[:, 4 * kg : 4 * (kg + 1),
                                128 * t : 128 * (t + 1)],
                            ps[:].rearrange("p (i r) -> p i r", i=4),
                            ninv4[:, t, :].rearrange("p (i r) -> p i r", i=4),
                            OP.mult,
                        )
                # fp8 quantize; ship shard kg
                for kg in range(8):
                    aq = p0.tile([128, 4, R], dt.float8e4, tag="aq", bufs=2,
                                 name=f"aq{kg}")
                    nc.scalar.activation(
                        aq[:].rearrange("p i r -> p (i r)"),
                        xnT[:, 4 * kg : 4 * (kg + 1), :]
                        .rearrange("p i r -> p (i r)"),
                        ACT.Identity, scale=QS,
                    )
                    nc.scalar.dma_start(
                        a2a_in[kg].rearrange("kk p r -> p kk r"), aq[:]
                    )
                cc("AllToAll", OP.bypass, a2a_in, a2a_out)

                # q partial: per 128-feat chunk, [128 feat x 42] (d0 | d1)
                q_sb = p0.tile([128, DT, 2 * K], dt.float32)
                for dtl in range(DT):
                    ps = pp_sm.tile([128, 2 * K], dt.float32, tag="sm", name=f"qp{dtl}")
                    for t in range(RT):
                        h = t // 2
                        nc.tensor.matmul(
                            ps[:, K * h : K * (h + 1)],
                            xrow[:, t, 128 * dtl : 128 * (dtl + 1)],
                            ohdiv[:, t, :], start=(t % 2 == 0), stop=(t % 2 == 1),
                        )
                    nc.vector.tensor_copy(q_sb[:, dtl, :], ps[:])
                for h in range(2):
                    nc.gpsimd.dma_start(
                        q_in[h].rearrange("k p j -> p k j"),
                        q_sb[:, :, K * h : K * (h + 1)],
                    )
                cc("ReduceScatter", OP.add, q_in, q_rs)

            # ============ centroid/XW1/Gram super-phase =====================
            # gsp holds a2s + gram staging (own region, no WAR with cend/w1p)
            with tc.tile_pool(name="gsp", bufs=1) as gsp:
                a2s = gsp.tile([128, 4, NC, R], dt.float8e4)
                with tc.tile_pool(name="cend", bufs=1) as cend:
                    # q shard -> floored bf16 centroid shard -> AllGather
                    qg = cend.tile([128, GQ, K], dt.float32)
                    nc.gpsimd.dma_start(
                        qg[:], q_rs[:].rearrange("g p j -> p g j")
                    )
                    qf = qg[:].rearrange("p g j -> p (g j)")
                    cti = cend.tile([128, GQ * K], dt.int32)
                    nc.vector.tensor_copy(cti[:], qf)
                    ctf = cend.tile([128, GQ * K], dt.float32)
                    nc.vector.tensor_copy(ctf[:], cti[:])
                    ltq = cend.tile([128, GQ * K], dt.float32)
                    nc.vector.tensor_tensor(ltq[:], qf, ctf[:], OP.is_lt)
                    ctfl = cend.tile([128, GQ * K], dt.float32)
                    nc.vector.tensor_tensor(ctfl[:], ctf[:], ltq[:], OP.subtract)
                    ctb = cend.tile([128, GQ, K], dt.bfloat16)
                    nc.vector.tensor_copy(
                        ctb[:].rearrange("p g j -> p (g j)"), ctfl[:]
                    )
                    nc.gpsimd.dma_start(
                        ctq_in[:].rearrange("g p j -> p g j"), ctb[:]
                    )
                    cc("AllGather", OP.bypass, ctq_in, ct_all)
                    ct_sb = cend.tile([128, 2 * DT, K], dt.bfloat16)
                    nc.gpsimd.dma_start(
                        ct_sb[:], ct_all[:].rearrange("g p j -> p g j")
                    )

                    # per-domain centroid norms via ones-matmul over ct^2
                    ct2 = cend.tile([128, 2 * DT * K], dt.float32)
                    nc.vector.tensor_tensor(
                        ct2[:], ct_sb[:].rearrange("p g j -> p (g j)"),
                        ct_sb[:].rearrange("p g j -> p (g j)"), OP.mult,
                    )
                    cnp = cend.tile([1, 2 * DT * K], dt.float32)
                    for hh in range(3):
                        w = 448
                        ps = pp_sm.tile([1, w], dt.float32, tag="sm",
                                        name=f"cn{hh}")
                        nc.tensor.matmul(
                            ps[:], ones_c[:], ct2[:, w * hh : w * (hh + 1)],
                            start=True, stop=True,
                        )
                        nc.vector.tensor_copy(cnp[:, w * hh : w * (hh + 1)],
                                              ps[:])
                    cn = cend.tile([1, 2, K, 1], dt.float32)
                    nc.vector.reduce_sum(
                        cn[:],
                        cnp[:].rearrange("p (d k j) -> p d j k", d=2, k=DT),
                        axis=AX,
                    )
                    cnv = cn[:].rearrange("p d j one -> p (d j one)")
                    nc.scalar.activation(cnv, cnv, ACT.Sqrt)
                    nc.vector.tensor_scalar(cnv, cnv, EPS, None, OP.max)
                    nc.gpsimd.partition_broadcast(
                        cnb[:].rearrange("p d j -> p (d j)"), cnv
                    )

                    # ---- XW1 = (x @ W1) via xnT, un-normalized at evac ----
                    with tc.tile_pool(name="w1p", bufs=1) as w1p:
                        for q in range(8):
                            w1q = w1p.tile([128, DT, 256], dt.bfloat16,
                                           tag="w1q", bufs=2, name=f"w1q{q}")
                            nc.sync.dma_start(
                                w1q[:],
                                w1_d.rearrange("(k p) f -> p k f", p=128)
                                [:, :, 256 * q : 256 * (q + 1)],
                            )
                            for t in range(RT):
                                ps = pp_g.tile([128, 256], dt.float32, tag="gc",
                                               name=f"xw_{q}_{t}")
                                for k in range(DT):
                                    nc.tensor.matmul(
                                        ps[:],
                                        xnT[:, k, 128 * t : 128 * (t + 1)],
                                        w1q[:, k, :],
                                        start=(k == 0), stop=(k == DT - 1),
                                    )
                                nc.scalar.activation(
                                    XW1[:, t, 256 * q : 256 * (q + 1)],
                                    ps[:], ACT.Identity,
                                    scale=norm_r[:, t : t + 1],
                                )

                    # ---- Zn = xn @ ct_dom ; simi per own tile ----
                    for t in range(RT):
                        dom = t // 2
                        ps = pp_sm.tile([128, K], dt.float32, tag="sm",
                                        name=f"zn{t}")
                        for k in range(DT):
                            nc.tensor.matmul(
                                ps[:], xnT[:, k, 128 * t : 128 * (t + 1)],
                                ct_sb[:, DT * dom + k, :],
                                start=(k == 0), stop=(k == DT - 1),
                            )
                        sel = cend.tile([128, K], dt.float32, tag="sel", bufs=2,
                                        name=f"sel{t}")
                        nc.vector.tensor_tensor(sel[:], ps[:], ohc[:, t, :],
                                                OP.mult)
                        num = cend.tile([128, 1], dt.float32, tag="num", bufs=2,
                                        name=f"num{t}")
                        nc.vector.reduce_sum(num[:], sel[:], axis=AX,
                                             apply_absolute_value=True)
                        den = cend.tile([128, K], dt.float32, tag="den", bufs=2,
                                        name=f"den{t}")
                        nc.vector.tensor_tensor(den[:], ohc[:, t, :],
                                                cnb[:, dom, :], OP.mult)
                        dens = cend.tile([128, 1], dt.float32, tag="dens",
                                         bufs=2, name=f"dens{t}")
                        nc.vector.reduce_sum(dens[:], den[:], axis=AX)
                        nc.vector.tensor_scalar(dens[:], dens[:], EPS, None,
                                                OP.max)
                        nc.vector.reciprocal(dens[:], dens[:])
                        nc.vector.tensor_tensor(simi[:, t : t + 1], num[:],
                                                dens[:], OP.mult)
                        nc.gpsimd.dma_start(
                            wv_in[:].rearrange("(t p one) -> t p one",
                                               t=RT, one=1)[t],
                            simi[:, t : t + 1],
                        )
                    cc("AllGather", OP.bypass, wv_in, wv_all)
                    nc.gpsimd.dma_start(
                        simi_all[:], wv_all[:].rearrange("(g p) -> p g", p=128)
                    )

                # ---- Gram: S12 & S21 partials from fp8 slices ----
                for s in range(NC):
                    nc.sync.dma_start(
                        a2s[:, :, s, :], a2a_out[s].rearrange("kk p r -> p kk r")
                    )
                # column-halved: ch=0 covers opposite-cols 0:1024, ch=1 rest.
                # hs=0: S12 tiles (d0 rows x d1 cols); hs=1: S21 (swapped mm).
                for ch in range(2):
                    for hs in range(2):
                        lo = 256 * hs          # lhsT row-half offset
                        ro = 256 * (1 - hs)    # rhs col-half offset
                        for T in range(16):
                            rr, e = T // 2, T % 2
                            st = gsp.tile([128, 1024], dt.bfloat16, tag="gst",
                                          bufs=3, name=f"gs{ch}_{hs}_{T}")
                            for Ci in range(2):
                                C = 2 * ch + Ci
                                ps = pp_g.tile([128, 512], dt.float32, tag="gc",
                                               name=f"g{ch}_{hs}_{T}_{Ci}")
                                for kk in range(4):
                                    nc.tensor.matmul(
                                        ps[:],
                                        a2s[:, kk, rr,
                                            lo + 128 * e : lo + 128 * (e + 1)],
                                        a2s[:, kk, 2 * C : 2 * C + 2,
                                            ro : ro + 256],
                                        start=(kk == 0), stop=(kk == 3),
                                    )
                                nc.scalar.activation(
                                    st[:, 512 * Ci : 512 * (Ci + 1)], ps[:],
                                    ACT.Identity, scale=1.0 / (QS * QS),
                                )
                            r0 = 512 * rr + 256 * hs + 128 * e
                            nc.scalar.dma_start(
                                rsg_in[ch][r0 : r0 + 128, :], st[:]
                            )
                    cc("ReduceScatter", OP.add, rsg_in[ch], rsg_out[ch])

            # ============ class max + w (after Gram so PE never stalls) ======
            with tc.tile_pool(name="wcalc", bufs=1) as wc:
                ohall = wc.tile([128, N // 128, K], dt.float32)
                nc.sync.dma_start(
                    ohall[:], ohall_d.rearrange("(g p) k -> p g k", p=128)
                )
                msk = wc.tile([128, N // 128, K], dt.float32)
                for g in range(N // 128):
                    nc.vector.tensor_scalar_mul(
                        msk[:, g, :], ohall[:, g, :], simi_all[:, g : g + 1]
                    )
                for dom in range(2):
                    gl = [4 * r + 2 * dom + e for r in range(NC) for e in range(2)]
                    mred = wc.tile([128, K], dt.float32, tag="mred", bufs=2,
                                   name=f"mred{dom}")
                    nc.vector.tensor_tensor(
                        mred[:], msk[:, gl[0], :], msk[:, gl[1], :], OP.max
                    )
                    for g in gl[2:]:
                        nc.vector.tensor_tensor(
                            mred[:], mred[:], msk[:, g, :], OP.max
                        )
                    pst = pp_t.tile([K, 128], dt.float32, tag="tp", name=f"cmt{dom}")
                    nc.tensor.transpose(pst[:], mred[:], eye_f[:])
                    cml = wc.tile([K, 1], dt.float32, tag="cml", bufs=2,
                                  name=f"cml{dom}")
                    nc.vector.reduce_max(cml[:], pst[:], axis=AX)
                    pmt = pp_t.tile([1, K], dt.float32, tag="tp", name=f"cmb{dom}")
                    nc.tensor.transpose(pmt[:], cml[:], eye_f[0:K, 0:K])
                    cmx = wc.tile([1, K], dt.float32, tag="cmx", bufs=2,
                                  name=f"cmx{dom}")
                    nc.vector.tensor_copy(cmx[:], pmt[:])
                    isz = wc.tile([1, K], dt.float32, tag="isz", bufs=2,
                                  name=f"isz{dom}")
                    nc.vector.tensor_scalar(isz[:], cmx[:], 0.0, None, OP.is_equal)
                    nc.vector.tensor_tensor(cmx[:], cmx[:], isz[:], OP.add)
                    nc.gpsimd.partition_broadcast(cmxb[:, dom, :], cmx[:])

                # w for own rows
                for t in range(RT):
                    mxs = wc.tile([128, K], dt.float32, tag="mxs", bufs=2,
                                  name=f"mxs{t}")
                    nc.vector.tensor_tensor(mxs[:], ohc[:, t, :],
                                            cmxb[:, t // 2, :], OP.mult)
                    mxv = wc.tile([128, 1], dt.float32, tag="mxv", bufs=2,
                                  name=f"mxv{t}")
                    nc.vector.reduce_sum(mxv[:], mxs[:], axis=AX)
                    nc.vector.reciprocal(mxv[:], mxv[:])
                    nc.vector.tensor_tensor(w_own[:, t : t + 1],
                                            simi[:, t : t + 1], mxv[:], OP.mult)

                # w for all rows -> broadcast rows per opposite-domain col order
                w_all = wc.tile([128, N // 128], dt.float32)
                for g in range(N // 128):
                    dom = (g % 4) // 2
                    mxs = wc.tile([128, K], dt.float32, tag="mxs", bufs=2,
                                  name=f"wmx{g}")
                    nc.vector.tensor_tensor(mxs[:], ohall[:, g, :],
                                            cmxb[:, dom, :], OP.mult)
                    mxv = wc.tile([128, 1], dt.float32, tag="mxv", bufs=2,
                                  name=f"wmv{g}")
                    nc.vector.reduce_sum(mxv[:], mxs[:], axis=AX)
                    nc.vector.reciprocal(mxv[:], mxv[:])
                    nc.vector.tensor_tensor(w_all[:, g : g + 1],
                                            simi_all[:, g : g + 1], mxv[:], OP.mult)
                wj0 = wc.tile([1, 2048], dt.float32)
                wj1 = wc.tile([1, 2048], dt.float32)
                for r in range(NC):
                    for e in range(2):
                        for dom in range(2):
                            g = 4 * r + 2 * dom + e
                            pw = pp_t.tile([1, 128], dt.float32, tag="tp",
                                           name=f"wt{g}")
                            nc.tensor.transpose(pw[:], w_all[:, g : g + 1], eye_f[:])
                            dst = wj0 if dom == 0 else wj1
                            nc.vector.tensor_copy(
                                dst[:, 256 * r + 128 * e : 256 * r + 128 * (e + 1)],
                                pw[:],
                            )
                nc.gpsimd.partition_broadcast(wjb0[:], wj0[:])
                nc.gpsimd.partition_broadcast(wjb1[:], wj1[:])

            # ============ S row-block: Wt, deg, dinv; Y1' ====================
            with tc.tile_pool(name="sproc", bufs=1) as sp:
                degp = sp.tile([128, 2, RT], dt.float32)
                for ch in range(2):
                    sblk = sp.tile([128, RT, 1024], dt.bfloat16, tag="sblk",
                                   bufs=2, name=f"sblk{ch}")
                    nc.gpsimd.dma_start(
                        sblk[:],
                        rsg_out[ch][:].rearrange("(t p) c -> p t c", p=128),
                    )
                    for t in range(RT):
                        wjb_o = wjb1 if t < 2 else wjb0
                        wslice = wjb_o[:, 1024 * ch : 1024 * (ch + 1)]
                        sabs = sp.tile([128, 1024], dt.float32, tag="sabs",
                                       bufs=2, name=f"sa{ch}_{t}")
                        nc.scalar.activation(sabs[:], sblk[:, t, :], ACT.Abs)
                        wd = sp.tile([128, 1024], dt.float32, tag="wd", bufs=2,
                                     name=f"wd{ch}_{t}")
                        nc.vector.tensor_scalar(wd[:], wslice,
                                                w_own[:, t : t + 1],
                                                None, OP.subtract)
                        nc.scalar.activation(wd[:], wd[:], ACT.Abs)
                        u = sp.tile([128, 1024], dt.float32, tag="u", bufs=2,
                                    name=f"u{ch}_{t}")
                        nc.vector.tensor_tensor(u[:], wd[:], sabs[:], OP.mult)
                        nc.vector.tensor_tensor(u[:], sabs[:], u[:], OP.subtract)
                        nc.vector.reduce_sum(degp[:, ch, t : t + 1], u[:], axis=AX)
                        nc.vector.tensor_copy(
                            A_blk[:, t, 1024 * ch : 1024 * (ch + 1)], u[:]
                        )
                nc.vector.tensor_tensor(deg[:], degp[:, 0, :], degp[:, 1, :],
                                        OP.add)
                nc.vector.tensor_scalar_add(deg[:], deg[:], 1.0)
                nc.vector.reciprocal(dinv[:], deg[:])
                nc.scalar.activation(dinv[:], dinv[:], ACT.Sqrt)
                # Y1' = dinv * XW1 (in place)
                for t in range(RT):
                    nc.vector.tensor_scalar_mul(
                        XW1[:, t, :], XW1[:, t, :], dinv[:, t : t + 1]
                    )

            # ============ GCN1 partial + split RS1 + h1; H2 accumulation =====
            with tc.tile_pool(name="gcn1", bufs=1) as g1:
                w2sb = g1.tile([128, F1 // 128, F2], dt.bfloat16)
                nc.sync.dma_start(
                    w2sb[:], w2_d.rearrange("(k p) f -> p k f", p=128)
                )
                for fh in range(2):
                    for T in range(32):
                        rr, sub = T // 4, T % 4
                        h, e = sub // 2, sub % 2
                        tjs = (2 * (1 - h), 2 * (1 - h) + 1)
                        r0 = 512 * rr + 256 * h + 128 * e
                        st = g1.tile([128, 1024], dt.bfloat16, tag="g1st",
                                     bufs=3, name=f"s1_{fh}_{T}")
                        for fc in range(2):
                            ps = pp_g.tile([128, 512], dt.float32, tag="gc",
                                           name=f"p1_{fh}_{T}_{fc}")
                            for i, tj in enumerate(tjs):
                                nc.tensor.matmul(
                                    ps[:],
                                    A_blk[:, tj, 256 * rr + 128 * e :
                                          256 * rr + 128 * (e + 1)],
                                    XW1[:, tj, 1024 * fh + 512 * fc :
                                        1024 * fh + 512 * (fc + 1)],
                                    start=(i == 0), stop=(i == 1),
                                )
                            nc.vector.tensor_copy(
                                st[:, 512 * fc : 512 * (fc + 1)], ps[:]
                            )
                        nc.gpsimd.dma_start(rs1_in[fh][r0 : r0 + 128, :], st[:])
                    cc("ReduceScatter", OP.add, rs1_in[fh], rs1_out[fh])

                    # h1 for this f-half + H2 partial contraction
                    rs1o = g1.tile([128, RT, F1 // 2], dt.bfloat16, tag="rs1o",
                                   bufs=2, name=f"rs1o{fh}")
                    nc.sync.dma_start(
                        rs1o[:],
                        rs1_out[fh][:].rearrange("(t p) f -> p t f", p=128),
                    )
                    for t in range(RT):
                        hsum = g1.tile([128, F1 // 2], dt.float32, tag="hsum",
                                       bufs=2, name=f"hs{fh}_{t}")
                        nc.vector.tensor_tensor(
                            hsum[:], rs1o[:, t, :],
                            XW1[:, t, 1024 * fh : 1024 * (fh + 1)], OP.add
                        )
                        nc.vector.tensor_scalar_mul(hsum[:], hsum[:],
                                                    dinv[:, t : t + 1])
                        for fg in range(2):
                            pt = pp_t.tile([128, 512], dt.float32, tag="tp",
                                           name=f"h1t{fh}_{t}_{fg}")
                            for fl4 in range(4):
                                fl = 4 * fg + fl4
                                nc.tensor.transpose(
                                    pt[:, 128 * fl4 : 128 * (fl4 + 1)],
                                    hsum[:, 128 * fl : 128 * (fl + 1)],
                                    eye_f[:],
                                )
                            for fl4 in range(4):
                                ft = 8 * fh + 4 * fg + fl4
                                nc.scalar.activation(
                                    h1T[:, ft, 128 * t : 128 * (t + 1)],
                                    pt[:, 128 * fl4 : 128 * (fl4 + 1)],
                                    ACT.Relu, bias=b1t[:, ft : ft + 1],
                                )
                    # H2 partial: contract this half's 8 f-tiles
                    for s in range(RT):
                        for fc in range(2):
                            ps = pp_g.tile([128, 512], dt.float32, tag="gc",
                                           name=f"h2_{fh}_{s}_{fc}")
                            for fl in range(8):
                                ft = 8 * fh + fl
                                nc.tensor.matmul(
                                    ps[:], h1T[:, ft, 128 * s : 128 * (s + 1)],
                                    w2sb[:, ft, 512 * fc : 512 * (fc + 1)],
                                    start=(fl == 0), stop=(fl == 7),
                                )
                            if fh == 0:
                                nc.vector.tensor_copy(
                                    H2f32[:, s, 512 * fc : 512 * (fc + 1)], ps[:]
                                )
                            else:
                                nc.vector.tensor_tensor(
                                    H2f32[:, s, 512 * fc : 512 * (fc + 1)],
                                    H2f32[:, s, 512 * fc : 512 * (fc + 1)],
                                    ps[:], OP.add,
                                )

            # ============ GCN2 partial + RS2 + h2 ===========================
            with tc.tile_pool(name="gcn2", bufs=1) as g2:
                # H2 bf16 and Y2' = dinv * H2
                for s in range(RT):
                    nc.vector.tensor_scalar_mul(
                        H2[:, s, :], H2f32[:, s, :], dinv[:, s : s + 1]
                    )
                for T in range(32):
                    rr, sub = T // 4, T % 4
                    h, e = sub // 2, sub % 2
                    tjs = (2 * (1 - h), 2 * (1 - h) + 1)
                    r0 = 512 * rr + 256 * h + 128 * e
                    st = g2.tile([128, 1024], dt.bfloat16, tag="g2st", bufs=3,
                                 name=f"s2_{T}")
                    for fc in range(2):
                        ps = pp_g.tile([128, 512], dt.float32, tag="gc",
                                       name=f"p2_{T}_{fc}")
                        for i, tj in enumerate(tjs):
                            nc.tensor.matmul(
                                ps[:],
                                A_blk[:, tj, 256 * rr + 128 * e :
                                      256 * rr + 128 * (e + 1)],
                                H2[:, tj, 512 * fc : 512 * (fc + 1)],
                                start=(i == 0), stop=(i == 1),
                            )
                        nc.vector.tensor_copy(
                            st[:, 512 * fc : 512 * (fc + 1)], ps[:]
                        )
                    nc.gpsimd.dma_start(rs2_in[r0 : r0 + 128, :], st[:])
                cc("ReduceScatter", OP.add, rs2_in, rs2_out)

                rs2o = g2.tile([128, RT, F2], dt.bfloat16)
                nc.sync.dma_start(
                    rs2o[:], rs2_out[:].rearrange("(t p) f -> p t f", p=128)
                )
                for t in range(RT):
                    hsum = g2.tile([128, F2], dt.float32, tag="hs2", bufs=2,
                                   name=f"hs2_{t}")
                    nc.vector.tensor_tensor(hsum[:], rs2o[:, t, :], H2[:, t, :],
                                            OP.add)
                    nc.vector.tensor_scalar_mul(hsum[:], hsum[:],
                                                dinv[:, t : t + 1])
                    for fg in range(2):
                        pt = pp_t.tile([128, 512], dt.float32, tag="tp",
                                       name=f"h2t{t}_{fg}")
                        for fl4 in range(4):
                            ft = 4 * fg + fl4
                            nc.tensor.transpose(
                                pt[:, 128 * fl4 : 128 * (fl4 + 1)],
                                hsum[:, 128 * ft : 128 * (ft + 1)], eye_f[:]
                            )
                        for fl4 in range(4):
                            ft = 4 * fg + fl4
                            nc.scalar.activation(
                                h2T[:, ft, 128 * t : 128 * (t + 1)],
                                pt[:, 128 * fl4 : 128 * (fl4 + 1)],
                                ACT.Identity, bias=b2t[:, ft : ft + 1],
                            )

            # ============ classifier + log-softmax + NLL + loss AG ==========
            with tc.tile_pool(name="cls", bufs=1) as cls:
                fw1s = cls.tile([128, F2 // 128, F3], dt.bfloat16)
                nc.sync.dma_start(
                    fw1s[:], fw1_d.rearrange("(k p) f -> p k f", p=128)
                )
                fw2s = cls.tile([128, F3 // 128, F4], dt.bfloat16)
                nc.sync.dma_start(
                    fw2s[:], fw2_d.rearrange("(k p) f -> p k f", p=128)
                )
                fw3s = cls.tile([128, F4 // 128, K], dt.bfloat16)
                nc.sync.dma_start(
                    fw3s[:], fw3_d.rearrange("(k p) f -> p k f", p=128)
                )
                h3T = cls.tile([128, F3 // 128, R], dt.bfloat16)
                for f in range(F3 // 128):
                    ps = pp_g.tile([128, R], dt.float32, tag="gc", name=f"c1_{f}")
                    for k in range(F2 // 128):
                        nc.tensor.matmul(
                            ps[:], fw1s[:, k, 128 * f : 128 * (f + 1)],
                            h2T[:, k, :],
                            start=(k == 0), stop=(k == F2 // 128 - 1),
                        )
                    nc.scalar.activation(
                        h3T[:, f, :], ps[:], ACT.Relu, bias=fb1t[:, f : f + 1]
                    )
                h4T = cls.tile([128, F4 // 128, R], dt.bfloat16)
                for f in range(F4 // 128):
                    ps = pp_g.tile([128, R], dt.float32, tag="gc", name=f"c2_{f}")
                    for k in range(F3 // 128):
                        nc.tensor.matmul(
                            ps[:], fw2s[:, k, 128 * f : 128 * (f + 1)],
                            h3T[:, k, :],
                            start=(k == 0), stop=(k == F3 // 128 - 1),
                        )
                    nc.scalar.activation(
                        h4T[:, f, :], ps[:], ACT.Relu, bias=fb2t[:, f : f + 1]
                    )
                pl = pp_g.tile([K, R], dt.float32, tag="gc", name="lgp")
                for k in range(F4 // 128):
                    nc.tensor.matmul(
                        pl[:], fw3s[:, k, :], h4T[:, k, :],
                        start=(k == 0), stop=(k == F4 // 128 - 1),
                    )
                lgt = cls.tile([K, R], dt.float32)
                nc.scalar.activation(lgt[:], pl[:], ACT.Identity, bias=fb3[:])

                # batched log-softmax + NLL over all 4 row-tiles.
                # logits are O(1) so the max-subtraction is skipped.
                pacc = pp_sm.tile([1, 1], dt.float32, tag="acc", bufs=1,
                                  name="lacc")
                plg = pp_t.tile([128, 4 * K], dt.float32, tag="tp", name="plg")
                for t in range(RT):
                    nc.tensor.transpose(
                        plg[:, K * t : K * (t + 1)],
                        lgt[:, 128 * t : 128 * (t + 1)], eye_f[0:K, 0:K]
                    )
                lgr = cls.tile([128, RT, K], dt.float32)
                nc.vector.tensor_copy(
                    lgr[:].rearrange("p t j -> p (t j)"), plg[:]
                )
                ex = cls.tile([128, RT, K], dt.float32)
                nc.scalar.activation(
                    ex[:].rearrange("p t j -> p (t j)"),
                    lgr[:].rearrange("p t j -> p (t j)"), ACT.Exp
                )
                sumex = cls.tile([128, RT, 1], dt.float32)
                nc.vector.reduce_sum(sumex[:], ex[:], axis=AX)
                lse = cls.tile([128, RT], dt.float32)
                nc.scalar.activation(
                    lse[:], sumex[:].rearrange("p t one -> p (t one)"), ACT.Ln
                )
                selm = cls.tile([128, RT, K], dt.float32)
                nc.vector.tensor_tensor(
                    selm[:].rearrange("p t j -> p (t j)"),
                    lgr[:].rearrange("p t j -> p (t j)"),
                    ohc[:].rearrange("p t j -> p (t j)"), OP.mult
                )
                selv = cls.tile([128, RT, 1], dt.float32)
                nc.vector.reduce_sum(selv[:], selm[:], axis=AX)
                nll = cls.tile([128, RT], dt.float32)
                nc.vector.tensor_tensor(
                    nll[:], lse[:], selv[:].rearrange("p t one -> p (t one)"),
                    OP.subtract
                )
                nlls = cls.tile([128, 1], dt.float32)
                nc.vector.reduce_sum(nlls[:], nll[:], axis=AX)
                nc.tensor.matmul(pacc[:], ones_c[:], nlls[:], start=True,
                                 stop=True)
                lsum = cls.tile([1, 1], dt.float32)
                nc.vector.tensor_copy(lsum[:], pacc[:])
                nc.sync.dma_start(
                    ls_in[:].rearrange("(p one) -> p one", one=1), lsum[:]
                )
                cc("AllGather", OP.bypass, ls_in, ls_out)
                lf = cls.tile([1, NC], dt.float32)
                nc.gpsimd.dma_start(
                    lf[:], ls_out[:].rearrange("(one k) -> one k", one=1)
                )
                lr = cls.tile([1, 1], dt.float32)
                nc.vector.reduce_sum(lr[:], lf[:], axis=AX)
                nc.vector.tensor_scalar_mul(lr[:], lr[:], 1.0 / N)
                nc.sync.dma_start(loss_d[:], lr[:])

    nc.finalize()
    return nc


_NC_CACHE = None


def kernel(x1, x2, label1, label2, W1, b1, W2, b2,
           fw1, fb1, fw2, fb2, fw3, fb3):
    global _NC_CACHE
    from concourse.bass_utils import run_bass_kernel_spmd

    x1 = np.asarray(x1, np.float32)
    x2 = np.asarray(x2, np.float32)
    label = np.concatenate([np.asarray(label1), np.asarray(label2)]).astype(np.int64)

    oh = np.zeros((N, K), np.float32)
    oh[np.arange(N), label] = 1.0
    su1 = np.maximum(oh[:2048].sum(0), 1.0)
    su2 = np.maximum(oh[2048:].sum(0), 1.0)
    ohdiv = np.concatenate([oh[:2048] / su1, oh[2048:] / su2], 0).astype(np.float32)

    # interleaved global order: rank r owns x1[256r:256r+256] ++ x2[256r:...]
    perm = np.concatenate(
        [np.r_[256 * r : 256 * (r + 1), 2048 + 256 * r : 2048 + 256 * (r + 1)]
         for r in range(NC)]
    )
    oh_g = np.ascontiguousarray(oh[perm])

    bf = ml_dtypes.bfloat16
    w1b = np.asarray(W1, np.float32).astype(bf)
    w2b = np.asarray(W2, np.float32).astype(bf)
    fw1b = np.asarray(fw1, np.float32).astype(bf)
    fw2b = np.asarray(fw2, np.float32).astype(bf)
    fw3b = np.asarray(fw3, np.float32).astype(bf)
    b1t = np.ascontiguousarray(np.asarray(b1, np.float32).reshape(F1 // 128, 128).T)
    b2t = np.ascontiguousarray(np.asarray(b2, np.float32).reshape(F2 // 128, 128).T)
    fb1t = np.ascontiguousarray(np.asarray(fb1, np.float32).reshape(F3 // 128, 128).T)
    fb2t = np.ascontiguousarray(np.asarray(fb2, np.float32).reshape(F4 // 128, 128).T)
    fb3c = np.asarray(fb3, np.float32).reshape(K, 1)

    if _NC_CACHE is None:
        _NC_CACHE = _build()
    nc = _NC_CACHE

    x = np.concatenate([x1, x2], 0)
    in_maps = []
    for c in range(NC):
        rows = perm[R * c : R * (c + 1)]
        in_maps.append({
            "xb": np.ascontiguousarray(x[rows]),
            "ohc": np.ascontiguousarray(oh[rows]),
            "ohdiv": np.ascontiguousarray(ohdiv[rows]),
            "ohall": oh_g,
            "w1b": w1b, "w2b": w2b, "fw1b": fw1b, "fw2b": fw2b, "fw3b": fw3b,
            "b1t": b1t, "b2t": b2t, "fb1t": fb1t, "fb2t": fb2t, "fb3c": fb3c,
        })

    res = run_bass_kernel_spmd(nc, in_maps, list(range(NC)))
    return np.asarray(res.results[0]["loss"], np.float32).reshape(())


# revision 3
# speedup vs baseline: 1.0461x; 1.0461x over previous
"""CDGRL (gnn_message_passing) Trainium2 kernel — 8-core SPMD, v2.

Uniform (no partition-id branches) restructure of the baseline:
- Interleaved row ownership: core c owns x1[256c:256c+256] ++ x2[256c:256c+256],
  so every core's 512 rows are half domain-0 / half domain-1 and the program
  is identical on all cores.
- S phase: feature-sharded Gram. AllToAll distributes fp8(xn*64) feature
  slices; each core computes the full inter-domain S12/S21 partial for its
  512-feature slice; one bf16 ReduceScatter delivers each core its own
  S row-block [own 512 x opposite-domain 2048]. Replaces the 33.5MB xnT
  AllGather.
- GCN layers: partial A@Y products from own columns + ReduceScatter,
  with D^-1/2 folded into local pre/post scalings (Y' = dinv*Y before the
  matmul, dinv_r applied post-RS on the owning core) so no degree exchange
  is needed at all. Self-loop diagonal added locally post-RS.
- Tiny AllGathers replace the cm/wv/deg/loss AllReduces.
Collective out-bytes drop ~8x vs baseline; no tc.If so TimelineSim works.
"""

import numpy as np
import ml_dtypes

N = 4096
D = 4096
K = 21
NC = 8
R = 512           # own rows per core (2 halves of 256: d0 then d1)
RT = 4            # 128-row tiles per core (t 0,1 = d0; t 2,3 = d1)
DT = 32           # 128-chunks of D
EPS = 1e-8
F1 = 2048
F2 = 1024
F3 = 512
F4 = 256
QS = 64.0         # fp8 quant scale for normalized rows
GQ = 8            # (dom,k) pairs per core in the q ReduceScatter shard


def _build():
    import concourse.bass as bass
    import concourse.mybir as mybir
    import concourse.tile as tile
    from concourse import bacc
    from concourse.masks import make_identity

    dt = mybir.dt
    AX = mybir.AxisListType.X
    OP = mybir.AluOpType
    ACT = mybir.ActivationFunctionType

    nc = bacc.Bacc("TRN2", target_bir_lowering=False, debug=False, num_devices=NC)

    xb = nc.dram_tensor("xb", [R, D], dt.float32, kind="ExternalInput")
    ohc_d = nc.dram_tensor("ohc", [R, K], dt.float32, kind="ExternalInput")
    ohdiv_d = nc.dram_tensor("ohdiv", [R, K], dt.float32, kind="ExternalInput")
    ohall_d = nc.dram_tensor("ohall", [N, K], dt.float32, kind="ExternalInput")
    w1_d = nc.dram_tensor("w1b", [D, F1], dt.bfloat16, kind="ExternalInput")
    w2_d = nc.dram_tensor("w2b", [F1, F2], dt.bfloat16, kind="ExternalInput")
    fw1_d = nc.dram_tensor("fw1b", [F2, F3], dt.bfloat16, kind="ExternalInput")
    fw2_d = nc.dram_tensor("fw2b", [F3, F4], dt.bfloat16, kind="ExternalInput")
    fw3_d = nc.dram_tensor("fw3b", [F4, K], dt.bfloat16, kind="ExternalInput")
    b1t_d = nc.dram_tensor("b1t", [128, F1 // 128], dt.float32, kind="ExternalInput")
    b2t_d = nc.dram_tensor("b2t", [128, F2 // 128], dt.float32, kind="ExternalInput")
    fb1t_d = nc.dram_tensor("fb1t", [128, F3 // 128], dt.float32, kind="ExternalInput")
    fb2t_d = nc.dram_tensor("fb2t", [128, F4 // 128], dt.float32, kind="ExternalInput")
    fb3_d = nc.dram_tensor("fb3c", [K, 1], dt.float32, kind="ExternalInput")
    loss_d = nc.dram_tensor("loss", [1, 1], dt.float32, kind="ExternalOutput")

    with tile.TileContext(nc) as tc:
        with (
            tc.tile_pool(name="dram", bufs=1, space="DRAM") as dram,
            tc.tile_pool(name="pers", bufs=1) as pers,
            tc.tile_pool(name="pp_g", bufs=3, space="PSUM") as pp_g,
            tc.tile_pool(name="pp_t", bufs=2, space="PSUM") as pp_t,
            tc.tile_pool(name="pp_sm", bufs=2, space="PSUM") as pp_sm,
        ):
            # ---- collective DRAM buffers ----
            q_in = dram.tile([2, DT, 128, K], dt.float32)
            q_rs = dram.tile([GQ, 128, K], dt.float32)
            ctq_in = dram.tile([GQ, 128, K], dt.bfloat16)
            ct_all = dram.tile([2 * DT, 128, K], dt.bfloat16)
            a2a_in = dram.tile([NC, 4, 128, R], dt.float8e4)
            a2a_out = dram.tile([NC, 4, 128, R], dt.float8e4)
            wv_in = dram.tile([R], dt.float32)
            wv_all = dram.tile([N], dt.float32, addr_space="Shared")
            rsg_in = [dram.tile([N, 1024], dt.bfloat16, name=f"rsgin{i}") for i in range(2)]
            rsg_out = [dram.tile([R, 1024], dt.bfloat16, name=f"rsgout{i}") for i in range(2)]
            rs1_in = [dram.tile([N, F1 // 2], dt.bfloat16, name=f"rs1in{i}") for i in range(2)]
            rs1_out = [dram.tile([R, F1 // 2], dt.bfloat16, name=f"rs1out{i}") for i in range(2)]
            rs2_in = dram.tile([N, F2], dt.bfloat16)
            rs2_out = dram.tile([R, F2], dt.bfloat16)
            ls_in = dram.tile([1], dt.float32)
            ls_out = dram.tile([NC], dt.float32, addr_space="Shared")

            GRP_ALL = [list(range(NC))]

            def cc(kind, op, i, o):
                nc.gpsimd.collective_compute(
                    kind, op, replica_groups=GRP_ALL, ins=[i.opt()], outs=[o.opt()]
                )

            # ---- persistent SBUF ----
            eye_f = pers.tile([128, 128], dt.float32)
            make_identity(nc, eye_f[:])
            ones_c = pers.tile([128, 1], dt.float32)
            nc.vector.memset(ones_c[:], 1.0)
            ohc = pers.tile([128, RT, K], dt.float32)
            nc.sync.dma_start(ohc[:], ohc_d.rearrange("(t p) k -> p t k", p=128))
            ohdiv = pers.tile([128, RT, K], dt.float32)
            nc.sync.dma_start(ohdiv[:], ohdiv_d.rearrange("(t p) k -> p t k", p=128))
            b1t = pers.tile([128, F1 // 128], dt.float32)
            nc.sync.dma_start(b1t[:], b1t_d[:])
            b2t = pers.tile([128, F2 // 128], dt.float32)
            nc.sync.dma_start(b2t[:], b2t_d[:])
            fb1t = pers.tile([128, F3 // 128], dt.float32)
            nc.sync.dma_start(fb1t[:], fb1t_d[:])
            fb2t = pers.tile([128, F4 // 128], dt.float32)
            nc.sync.dma_start(fb2t[:], fb2t_d[:])
            fb3 = pers.tile([K, 1], dt.float32)
            nc.sync.dma_start(fb3[:], fb3_d[:])

            xnT = pers.tile([128, DT, R], dt.bfloat16)
            XW1 = pers.tile([128, RT, F1], dt.bfloat16)   # becomes Y1' in place
            A_blk = pers.tile([128, RT, 2048], dt.bfloat16)
            h1T = pers.tile([128, F1 // 128, R], dt.bfloat16)
            H2f32 = pers.tile([128, RT, F2], dt.float32)
            H2 = pers.tile([128, RT, F2], dt.bfloat16)    # Y2' = dinv*H2
            h2T = pers.tile([128, F2 // 128, R], dt.bfloat16)
            cnb = pers.tile([128, 2, K], dt.float32)
            cmxb = pers.tile([128, 2, K], dt.float32)
            wjb0 = pers.tile([128, 2048], dt.float32)     # w over d0 cols
            wjb1 = pers.tile([128, 2048], dt.float32)     # w over d1 cols
            w_own = pers.tile([128, RT], dt.float32)
            simi = pers.tile([128, RT], dt.float32)
            simi_all = pers.tile([128, N // 128], dt.float32)
            norm_r = pers.tile([128, RT], dt.float32)
            norm_b = pers.tile([128, RT], dt.float32)
            norm_c = pers.tile([128, RT], dt.float32)
            norm_d = pers.tile([128, RT], dt.float32)
            ninv_r = pers.tile([128, RT], dt.float32)
            ninvj = pers.tile([1, R], dt.float32)
            ninvjb = pers.tile([128, R], dt.float32)
            dinv = pers.tile([128, RT], dt.float32)
            deg = pers.tile([128, RT], dt.float32)

            normq = [norm_r, norm_b, norm_c, norm_d]

            # ============ P0: x load, norms, xn transpose, fp8 ship, q ======
            with tc.tile_pool(name="p0", bufs=1) as p0:
                xrow = p0.tile([128, RT, D], dt.float32)
                xbv = xb.rearrange("(t p) d -> p t d", p=128)
                for t in range(RT):
                    nc.sync.dma_start(xrow[:, t, :], xbv[:, t, :])

                # row norms via ACT Square accumulate (4 quarter-D passes/tile)
                for t in range(RT):
                    for hh in range(4):
                        sq = p0.tile([128, D // 4], dt.float32, tag="sq", bufs=1,
                                     name=f"sq{t}_{hh}")
                        nc.scalar.activation(
                            sq[:], xrow[:, t, (D // 4) * hh : (D // 4) * (hh + 1)],
                            ACT.Square,
                            accum_out=normq[hh][:, t : t + 1],
                        )
                for hh in range(1, 4):
                    nc.vector.tensor_tensor(norm_r[:], norm_r[:], normq[hh][:],
                                            OP.add)
                nc.scalar.activation(norm_r[:], norm_r[:], ACT.Sqrt)
                nc.vector.tensor_scalar(ninv_r[:], norm_r[:], EPS, None, OP.max)
                nc.vector.reciprocal(ninv_r[:], ninv_r[:])
                for t in range(RT):
                    pw = pp_sm.tile([1, 128], dt.float32, tag="sm", name=f"nv{t}")
                    nc.tensor.transpose(pw[:], ninv_r[:, t : t + 1], eye_f[:])
                    nc.vector.tensor_copy(ninvj[:, 128 * t : 128 * (t + 1)], pw[:])
                nc.gpsimd.partition_broadcast(ninvjb[:], ninvj[:])

                # transpose raw x (batched 4-wide into one PSUM bank);
                # normalize during evacuation with repeated-ninv rows
                ninv4 = p0.tile([128, RT, 512], dt.float32)
                for t in range(RT):
                    for i in range(4):
                        nc.vector.tensor_copy(
                            ninv4[:, t, 128 * i : 128 * (i + 1)],
                            ninvjb[:, 128 * t : 128 * (t + 1)],
                        )
                for t in range(RT):
                    for kg in range(8):
                        ps = pp_t.tile([128, 512], dt.float32, tag="tp",
                                       name=f"tp{kg}_{t}")
                        for i in range(4):
                            nc.tensor.transpose(
                                ps[:, 128 * i : 128 * (i + 1)],
                                xrow[:, t, 512 * kg + 128 * i :
                                     512 * kg + 128 * (i + 1)],
                                eye_f[:],
                            )
                        nc.vector.tensor_tensor(
                            xnT[:, 4 * kg : 4 * (kg + 1),
                                128 * t : 128 * (t + 1)],
                            ps[:].rearrange("p (i r) -> p i r", i=4),
                            ninv4[:, t, :].rearrange("p (i r) -> p i r", i=4),
                            OP.mult,
                        )
                # fp8 quantize; ship shard kg
                for kg in range(8):
                    aq = p0.tile([128, 4, R], dt.float8e4, tag="aq", bufs=2,
                                 name=f"aq{kg}")
                    nc.scalar.activation(
                        aq[:].rearrange("p i r -> p (i r)"),
                        xnT[:, 4 * kg : 4 * (kg + 1), :]
                        .rearrange("p i r -> p (i r)"),
                        ACT.Identity, scale=QS,
                    )
                    nc.scalar.dma_start(
                        a2a_in[kg].rearrange("kk p r -> p kk r"), aq[:]
                    )
                cc("AllToAll", OP.bypass, a2a_in, a2a_out)

                # q partial: per 128-feat chunk, [128 feat x 42] (d0 | d1)
                q_sb = p0.tile([128, DT, 2 * K], dt.float32)
                for dtl in range(DT):
                    ps = pp_sm.tile([128, 2 * K], dt.float32, tag="sm", name=f"qp{dtl}")
                    for t in range(RT):
                        h = t // 2
                        nc.tensor.matmul(
                            ps[:, K * h : K * (h + 1)],
                            xrow[:, t, 128 * dtl : 128 * (dtl + 1)],
                            ohdiv[:, t, :], start=(t % 2 == 0), stop=(t % 2 == 1),
                        )
                    nc.vector.tensor_copy(q_sb[:, dtl, :], ps[:])
                for h in range(2):
                    nc.gpsimd.dma_start(
                        q_in[h].rearrange("k p j -> p k j"),
                        q_sb[:, :, K * h : K * (h + 1)],
                    )
                cc("ReduceScatter", OP.add, q_in, q_rs)

            # ============ centroid/XW1/Gram super-phase =====================
            # gsp holds a2s + gram staging (own region, no WAR with cend/w1p)
            with tc.tile_pool(name="gsp", bufs=1) as gsp:
                a2s = gsp.tile([128, 4, NC, R], dt.float8e4)
                with tc.tile_pool(name="cend", bufs=1) as cend:
                    # q shard -> floored bf16 centroid shard -> AllGather
                    qg = cend.tile([128, GQ, K], dt.float32)
                    nc.gpsimd.dma_start(
                        qg[:], q_rs[:].rearrange("g p j -> p g j")
                    )
                    qf = qg[:].rearrange("p g j -> p (g j)")
                    cti = cend.tile([128, GQ * K], dt.int32)
                    nc.vector.tensor_copy(cti[:], qf)
                    ctf = cend.tile([128, GQ * K], dt.float32)
                    nc.vector.tensor_copy(ctf[:], cti[:])
                    ltq = cend.tile([128, GQ * K], dt.float32)
                    nc.vector.tensor_tensor(ltq[:], qf, ctf[:], OP.is_lt)
                    ctfl = cend.tile([128, GQ * K], dt.float32)
                    nc.vector.tensor_tensor(ctfl[:], ctf[:], ltq[:], OP.subtract)
                    ctb = cend.tile([128, GQ, K], dt.bfloat16)
                    nc.vector.tensor_copy(
                        ctb[:].rearrange("p g j -> p (g j)"), ctfl[:]
                    )
                    nc.gpsimd.dma_start(
                        ctq_in[:].rearrange("g p j -> p g j"), ctb[:]
                    )
                    cc("AllGather", OP.bypass, ctq_in, ct_all)
                    ct_sb = cend.tile([128, 2 * DT, K], dt.bfloat16)
                    nc.gpsimd.dma_start(
                        ct_sb[:], ct_all[:].rearrange("g p j -> p g j")
                    )

                    # ---- XW1 = (x @ W1) via xnT, un-normalized at evac ----
                    with tc.tile_pool(name="w1p", bufs=1) as w1p:
                        for q in range(8):
                            w1q = w1p.tile([128, DT, 256], dt.bfloat16,
                                           tag="w1q", bufs=2, name=f"w1q{q}")
                            nc.sync.dma_start(
                                w1q[:],
                                w1_d.rearrange("(k p) f -> p k f", p=128)
                                [:, :, 256 * q : 256 * (q + 1)],
                            )
                            for t in range(RT):
                                ps = pp_g.tile([128, 256], dt.float32, tag="gc",
                                               name=f"xw_{q}_{t}")
                                for k in range(DT):
                                    nc.tensor.matmul(
                                        ps[:],
                                        xnT[:, k, 128 * t : 128 * (t + 1)],
                                        w1q[:, k, :],
                                        start=(k == 0), stop=(k == DT - 1),
                                    )
                                nc.scalar.activation(
                                    XW1[:, t, 256 * q : 256 * (q + 1)],
                                    ps[:], ACT.Identity,
                                    scale=norm_r[:, t : t + 1],
                                )

                    # per-domain centroid norms via ones-matmul over ct^2
                    ct2 = cend.tile([128, 2 * DT * K], dt.float32)
                    nc.vector.tensor_tensor(
                        ct2[:], ct_sb[:].rearrange("p g j -> p (g j)"),
                        ct_sb[:].rearrange("p g j -> p (g j)"), OP.mult,
                    )
                    cnp = cend.tile([1, 2 * DT * K], dt.float32)
                    for hh in range(3):
                        w = 448
                        ps = pp_sm.tile([1, w], dt.float32, tag="sm",
                                        name=f"cn{hh}")
                        nc.tensor.matmul(
                            ps[:], ones_c[:], ct2[:, w * hh : w * (hh + 1)],
                            start=True, stop=True,
                        )
                        nc.vector.tensor_copy(cnp[:, w * hh : w * (hh + 1)],
                                              ps[:])
                    cn = cend.tile([1, 2, K, 1], dt.float32)
                    nc.vector.reduce_sum(
                        cn[:],
                        cnp[:].rearrange("p (d k j) -> p d j k", d=2, k=DT),
                        axis=AX,
                    )
                    cnv = cn[:].rearrange("p d j one -> p (d j one)")
                    nc.scalar.activation(cnv, cnv, ACT.Sqrt)
                    nc.vector.tensor_scalar(cnv, cnv, EPS, None, OP.max)
                    nc.gpsimd.partition_broadcast(
                        cnb[:].rearrange("p d j -> p (d j)"), cnv
                    )

                    # ---- Zn = xn @ ct_dom ; simi per own tile ----
                    for t in range(RT):
                        dom = t // 2
                        ps = pp_sm.tile([128, K], dt.float32, tag="sm",
                                        name=f"zn{t}")
                        for k in range(DT):
                            nc.tensor.matmul(
                                ps[:], xnT[:, k, 128 * t : 128 * (t + 1)],
                                ct_sb[:, DT * dom + k, :],
                                start=(k == 0), stop=(k == DT - 1),
                            )
                        sel = cend.tile([128, K], dt.float32, tag="sel", bufs=2,
                                        name=f"sel{t}")
                        nc.vector.tensor_tensor(sel[:], ps[:], ohc[:, t, :],
                                                OP.mult)
                        num = cend.tile([128, 1], dt.float32, tag="num", bufs=2,
                                        name=f"num{t}")
                        nc.vector.reduce_sum(num[:], sel[:], axis=AX,
                                             apply_absolute_value=True)
                        den = cend.tile([128, K], dt.float32, tag="den", bufs=2,
                                        name=f"den{t}")
                        nc.vector.tensor_tensor(den[:], ohc[:, t, :],
                                                cnb[:, dom, :], OP.mult)
                        dens = cend.tile([128, 1], dt.float32, tag="dens",
                                         bufs=2, name=f"dens{t}")
                        nc.vector.reduce_sum(dens[:], den[:], axis=AX)
                        nc.vector.tensor_scalar(dens[:], dens[:], EPS, None,
                                                OP.max)
                        nc.vector.reciprocal(dens[:], dens[:])
                        nc.vector.tensor_tensor(simi[:, t : t + 1], num[:],
                                                dens[:], OP.mult)
                        nc.gpsimd.dma_start(
                            wv_in[:].rearrange("(t p one) -> t p one",
                                               t=RT, one=1)[t],
                            simi[:, t : t + 1],
                        )
                    cc("AllGather", OP.bypass, wv_in, wv_all)
                    nc.gpsimd.dma_start(
                        simi_all[:], wv_all[:].rearrange("(g p) -> p g", p=128)
                    )

                # ---- Gram: S12 & S21 partials from fp8 slices ----
                for s in range(NC):
                    nc.sync.dma_start(
                        a2s[:, :, s, :], a2a_out[s].rearrange("kk p r -> p kk r")
                    )
                # column-halved: ch=0 covers opposite-cols 0:1024, ch=1 rest.
                # hs=0: S12 tiles (d0 rows x d1 cols); hs=1: S21 (swapped mm).
                for ch in range(2):
                    for hs in range(2):
                        lo = 256 * hs          # lhsT row-half offset
                        ro = 256 * (1 - hs)    # rhs col-half offset
                        for T in range(16):
                            rr, e = T // 2, T % 2
                            st = gsp.tile([128, 1024], dt.bfloat16, tag="gst",
                                          bufs=3, name=f"gs{ch}_{hs}_{T}")
                            for Ci in range(2):
                                C = 2 * ch + Ci
                                ps = pp_g.tile([128, 512], dt.float32, tag="gc",
                                               name=f"g{ch}_{hs}_{T}_{Ci}")
                                for kk in range(4):
                                    nc.tensor.matmul(
                                        ps[:],
                                        a2s[:, kk, rr,
                                            lo + 128 * e : lo + 128 * (e + 1)],
                                        a2s[:, kk, 2 * C : 2 * C + 2,
                                            ro : ro + 256],
                                        start=(kk == 0), stop=(kk == 3),
                                    )
                                nc.scalar.activation(
                                    st[:, 512 * Ci : 512 * (Ci + 1)], ps[:],
                                    ACT.Identity, scale=1.0 / (QS * QS),
                                )
                            r0 = 512 * rr + 256 * hs + 128 * e
                            nc.scalar.dma_start(
                                rsg_in[ch][r0 : r0 + 128, :], st[:]
                            )
                    cc("ReduceScatter", OP.add, rsg_in[ch], rsg_out[ch])

            # ============ class max + w (after Gram so PE never stalls) ======
            with tc.tile_pool(name="wcalc", bufs=1) as wc:
                ohall = wc.tile([128, N // 128, K], dt.float32)
                nc.sync.dma_start(
                    ohall[:], ohall_d.rearrange("(g p) k -> p g k", p=128)
                )
                msk = wc.tile([128, N // 128, K], dt.float32)
                for g in range(N // 128):
                    nc.vector.tensor_scalar_mul(
                        msk[:, g, :], ohall[:, g, :], simi_all[:, g : g + 1]
                    )
                for dom in range(2):
                    gl = [4 * r + 2 * dom + e for r in range(NC) for e in range(2)]
                    mred = wc.tile([128, K], dt.float32, tag="mred", bufs=2,
                                   name=f"mred{dom}")
                    nc.vector.tensor_tensor(
                        mred[:], msk[:, gl[0], :], msk[:, gl[1], :], OP.max
                    )
                    for g in gl[2:]:
                        nc.vector.tensor_tensor(
                            mred[:], mred[:], msk[:, g, :], OP.max
                        )
                    pst = pp_t.tile([K, 128], dt.float32, tag="tp", name=f"cmt{dom}")
                    nc.tensor.transpose(pst[:], mred[:], eye_f[:])
                    cml = wc.tile([K, 1], dt.float32, tag="cml", bufs=2,
                                  name=f"cml{dom}")
                    nc.vector.reduce_max(cml[:], pst[:], axis=AX)
                    pmt = pp_t.tile([1, K], dt.float32, tag="tp", name=f"cmb{dom}")
                    nc.tensor.transpose(pmt[:], cml[:], eye_f[0:K, 0:K])
                    cmx = wc.tile([1, K], dt.float32, tag="cmx", bufs=2,
                                  name=f"cmx{dom}")
                    nc.vector.tensor_copy(cmx[:], pmt[:])
                    isz = wc.tile([1, K], dt.float32, tag="isz", bufs=2,
                                  name=f"isz{dom}")
                    nc.vector.tensor_scalar(isz[:], cmx[:], 0.0, None, OP.is_equal)
                    nc.vector.tensor_tensor(cmx[:], cmx[:], isz[:], OP.add)
                    nc.gpsimd.partition_broadcast(cmxb[:, dom, :], cmx[:])

                # w for own rows
                for t in range(RT):
                    mxs = wc.tile([128, K], dt.float32, tag="mxs", bufs=2,
                                  name=f"mxs{t}")
                    nc.vector.tensor_tensor(mxs[:], ohc[:, t, :],
                                            cmxb[:, t // 2, :], OP.mult)
                    mxv = wc.tile([128, 1], dt.float32, tag="mxv", bufs=2,
                                  name=f"mxv{t}")
                    nc.vector.reduce_sum(mxv[:], mxs[:], axis=AX)
                    nc.vector.reciprocal(mxv[:], mxv[:])
                    nc.vector.tensor_tensor(w_own[:, t : t + 1],
                                            simi[:, t : t + 1], mxv[:], OP.mult)

                # w for all rows -> broadcast rows per opposite-domain col order
                w_all = wc.tile([128, N // 128], dt.float32)
                for g in range(N // 128):
                    dom = (g % 4) // 2
                    mxs = wc.tile([128, K], dt.float32, tag="mxs", bufs=2,
                                  name=f"wmx{g}")
                    nc.vector.tensor_tensor(mxs[:], ohall[:, g, :],
                                            cmxb[:, dom, :], OP.mult)
                    mxv = wc.tile([128, 1], dt.float32, tag="mxv", bufs=2,
                                  name=f"wmv{g}")
                    nc.vector.reduce_sum(mxv[:], mxs[:], axis=AX)
                    nc.vector.reciprocal(mxv[:], mxv[:])
                    nc.vector.tensor_tensor(w_all[:, g : g + 1],
                                            simi_all[:, g : g + 1], mxv[:], OP.mult)
                wj0 = wc.tile([1, 2048], dt.float32)
                wj1 = wc.tile([1, 2048], dt.float32)
                for r in range(NC):
                    for e in range(2):
                        for dom in range(2):
                            g = 4 * r + 2 * dom + e
                            pw = pp_t.tile([1, 128], dt.float32, tag="tp",
                                           name=f"wt{g}")
                            nc.tensor.transpose(pw[:], w_all[:, g : g + 1], eye_f[:])
                            dst = wj0 if dom == 0 else wj1
                            nc.vector.tensor_copy(
                                dst[:, 256 * r + 128 * e : 256 * r + 128 * (e + 1)],
                                pw[:],
                            )
                nc.gpsimd.partition_broadcast(wjb0[:], wj0[:])
                nc.gpsimd.partition_broadcast(wjb1[:], wj1[:])

            # ============ S row-block: Wt, deg, dinv; Y1' ====================
            with tc.tile_pool(name="sproc", bufs=1) as sp:
                degp = sp.tile([128, 2, RT], dt.float32)
                for ch in range(2):
                    sblk = sp.tile([128, RT, 1024], dt.bfloat16, tag="sblk",
                                   bufs=2, name=f"sblk{ch}")
                    nc.gpsimd.dma_start(
                        sblk[:],
                        rsg_out[ch][:].rearrange("(t p) c -> p t c", p=128),
                    )
                    for t in range(RT):
                        wjb_o = wjb1 if t < 2 else wjb0
                        wslice = wjb_o[:, 1024 * ch : 1024 * (ch + 1)]
                        sabs = sp.tile([128, 1024], dt.float32, tag="sabs",
                                       bufs=2, name=f"sa{ch}_{t}")
                        nc.scalar.activation(sabs[:], sblk[:, t, :], ACT.Abs)
                        wd = sp.tile([128, 1024], dt.float32, tag="wd", bufs=2,
                                     name=f"wd{ch}_{t}")
                        nc.vector.tensor_scalar(wd[:], wslice,
                                                w_own[:, t : t + 1],
                                                None, OP.subtract)
                        nc.scalar.activation(wd[:], wd[:], ACT.Abs)
                        u = sp.tile([128, 1024], dt.float32, tag="u", bufs=2,
                                    name=f"u{ch}_{t}")
                        nc.vector.tensor_tensor(u[:], wd[:], sabs[:], OP.mult)
                        nc.vector.tensor_tensor(u[:], sabs[:], u[:], OP.subtract)
                        nc.vector.reduce_sum(degp[:, ch, t : t + 1], u[:], axis=AX)
                        nc.vector.tensor_copy(
                            A_blk[:, t, 1024 * ch : 1024 * (ch + 1)], u[:]
                        )
                nc.vector.tensor_tensor(deg[:], degp[:, 0, :], degp[:, 1, :],
                                        OP.add)
                nc.vector.tensor_scalar_add(deg[:], deg[:], 1.0)
                nc.vector.reciprocal(dinv[:], deg[:])
                nc.scalar.activation(dinv[:], dinv[:], ACT.Sqrt)
                # Y1' = dinv * XW1 (in place)
                for t in range(RT):
                    nc.vector.tensor_scalar_mul(
                        XW1[:, t, :], XW1[:, t, :], dinv[:, t : t + 1]
                    )

            # ============ GCN1 partial + split RS1 + h1; H2 accumulation =====
            with tc.tile_pool(name="gcn1", bufs=1) as g1:
                w2sb = g1.tile([128, F1 // 128, F2], dt.bfloat16)
                nc.sync.dma_start(
                    w2sb[:], w2_d.rearrange("(k p) f -> p k f", p=128)
                )
                for fh in range(2):
                    for T in range(32):
                        rr, sub = T // 4, T % 4
                        h, e = sub // 2, sub % 2
                        tjs = (2 * (1 - h), 2 * (1 - h) + 1)
                        r0 = 512 * rr + 256 * h + 128 * e
                        st = g1.tile([128, 1024], dt.bfloat16, tag="g1st",
                                     bufs=3, name=f"s1_{fh}_{T}")
                        for fc in range(2):
                            ps = pp_g.tile([128, 512], dt.float32, tag="gc",
                                           name=f"p1_{fh}_{T}_{fc}")
                            for i, tj in enumerate(tjs):
                                nc.tensor.matmul(
                                    ps[:],
                                    A_blk[:, tj, 256 * rr + 128 * e :
                                          256 * rr + 128 * (e + 1)],
                                    XW1[:, tj, 1024 * fh + 512 * fc :
                                        1024 * fh + 512 * (fc + 1)],
                                    start=(i == 0), stop=(i == 1),
                                )
                            nc.vector.tensor_copy(
                                st[:, 512 * fc : 512 * (fc + 1)], ps[:]
                            )
                        nc.gpsimd.dma_start(rs1_in[fh][r0 : r0 + 128, :], st[:])
                    cc("ReduceScatter", OP.add, rs1_in[fh], rs1_out[fh])

                    # h1 for this f-half + H2 partial contraction
                    rs1o = g1.tile([128, RT, F1 // 2], dt.bfloat16, tag="rs1o",
                                   bufs=2, name=f"rs1o{fh}")
                    nc.sync.dma_start(
                        rs1o[:],
                        rs1_out[fh][:].rearrange("(t p) f -> p t f", p=128),
                    )
                    for t in range(RT):
                        hsum = g1.tile([128, F1 // 2], dt.float32, tag="hsum",
                                       bufs=2, name=f"hs{fh}_{t}")
                        nc.vector.tensor_tensor(
                            hsum[:], rs1o[:, t, :],
                            XW1[:, t, 1024 * fh : 1024 * (fh + 1)], OP.add
                        )
                        nc.vector.tensor_scalar_mul(hsum[:], hsum[:],
                                                    dinv[:, t : t + 1])
                        for fg in range(2):
                            pt = pp_t.tile([128, 512], dt.float32, tag="tp",
                                           name=f"h1t{fh}_{t}_{fg}")
                            for fl4 in range(4):
                                fl = 4 * fg + fl4
                                nc.tensor.transpose(
                                    pt[:, 128 * fl4 : 128 * (fl4 + 1)],
                                    hsum[:, 128 * fl : 128 * (fl + 1)],
                                    eye_f[:],
                                )
                            for fl4 in range(4):
                                ft = 8 * fh + 4 * fg + fl4
                                nc.scalar.activation(
                                    h1T[:, ft, 128 * t : 128 * (t + 1)],
                                    pt[:, 128 * fl4 : 128 * (fl4 + 1)],
                                    ACT.Relu, bias=b1t[:, ft : ft + 1],
                                )
                    # H2 partial: contract this half's 8 f-tiles
                    for s in range(RT):
                        for fc in range(2):
                            ps = pp_g.tile([128, 512], dt.float32, tag="gc",
                                           name=f"h2_{fh}_{s}_{fc}")
                            for fl in range(8):
                                ft = 8 * fh + fl
                                nc.tensor.matmul(
                                    ps[:], h1T[:, ft, 128 * s : 128 * (s + 1)],
                                    w2sb[:, ft, 512 * fc : 512 * (fc + 1)],
                                    start=(fl == 0), stop=(fl == 7),
                                )
                            if fh == 0:
                                nc.vector.tensor_copy(
                                    H2f32[:, s, 512 * fc : 512 * (fc + 1)], ps[:]
                                )
                            else:
                                nc.vector.tensor_tensor(
                                    H2f32[:, s, 512 * fc : 512 * (fc + 1)],
                                    H2f32[:, s, 512 * fc : 512 * (fc + 1)],
                                    ps[:], OP.add,
                                )

            # ============ GCN2 partial + RS2 + h2 ===========================
            with tc.tile_pool(name="gcn2", bufs=1) as g2:
                # H2 bf16 and Y2' = dinv * H2
                for s in range(RT):
                    nc.vector.tensor_scalar_mul(
                        H2[:, s, :], H2f32[:, s, :], dinv[:, s : s + 1]
                    )
                for T in range(32):
                    rr, sub = T // 4, T % 4
                    h, e = sub // 2, sub % 2
                    tjs = (2 * (1 - h), 2 * (1 - h) + 1)
                    r0 = 512 * rr + 256 * h + 128 * e
                    st = g2.tile([128, 1024], dt.bfloat16, tag="g2st", bufs=3,
                                 name=f"s2_{T}")
                    for fc in range(2):
                        ps = pp_g.tile([128, 512], dt.float32, tag="gc",
                                       name=f"p2_{T}_{fc}")
                        for i, tj in enumerate(tjs):
                            nc.tensor.matmul(
                                ps[:],
                                A_blk[:, tj, 256 * rr + 128 * e :
                                      256 * rr + 128 * (e + 1)],
                                H2[:, tj, 512 * fc : 512 * (fc + 1)],
                                start=(i == 0), stop=(i == 1),
                            )
                        nc.vector.tensor_copy(
                            st[:, 512 * fc : 512 * (fc + 1)], ps[:]
                        )
                    nc.gpsimd.dma_start(rs2_in[r0 : r0 + 128, :], st[:])
                cc("ReduceScatter", OP.add, rs2_in, rs2_out)

                rs2o = g2.tile([128, RT, F2], dt.bfloat16)
                nc.sync.dma_start(
                    rs2o[:], rs2_out[:].rearrange("(t p) f -> p t f", p=128)
                )
                for t in range(RT):
                    hsum = g2.tile([128, F2], dt.float32, tag="hs2", bufs=2,
                                   name=f"hs2_{t}")
                    nc.vector.tensor_tensor(hsum[:], rs2o[:, t, :], H2[:, t, :],
                                            OP.add)
                    nc.vector.tensor_scalar_mul(hsum[:], hsum[:],
                                                dinv[:, t : t + 1])
                    for fg in range(2):
                        pt = pp_t.tile([128, 512], dt.float32, tag="tp",
                                       name=f"h2t{t}_{fg}")
                        for fl4 in range(4):
                            ft = 4 * fg + fl4
                            nc.tensor.transpose(
                                pt[:, 128 * fl4 : 128 * (fl4 + 1)],
                                hsum[:, 128 * ft : 128 * (ft + 1)], eye_f[:]
                            )
                        for fl4 in range(4):
                            ft = 4 * fg + fl4
                            nc.scalar.activation(
                                h2T[:, ft, 128 * t : 128 * (t + 1)],
                                pt[:, 128 * fl4 : 128 * (fl4 + 1)],
                                ACT.Identity, bias=b2t[:, ft : ft + 1],
                            )

            # ============ classifier + log-softmax + NLL + loss AG ==========
            with tc.tile_pool(name="cls", bufs=1) as cls:
                fw1s = cls.tile([128, F2 // 128, F3], dt.bfloat16)
                nc.sync.dma_start(
                    fw1s[:], fw1_d.rearrange("(k p) f -> p k f", p=128)
                )
                fw2s = cls.tile([128, F3 // 128, F4], dt.bfloat16)
                nc.sync.dma_start(
                    fw2s[:], fw2_d.rearrange("(k p) f -> p k f", p=128)
                )
                fw3s = cls.tile([128, F4 // 128, K], dt.bfloat16)
                nc.sync.dma_start(
                    fw3s[:], fw3_d.rearrange("(k p) f -> p k f", p=128)
                )
                h3T = cls.tile([128, F3 // 128, R], dt.bfloat16)
                for f in range(F3 // 128):
                    ps = pp_g.tile([128, R], dt.float32, tag="gc", name=f"c1_{f}")
                    for k in range(F2 // 128):
                        nc.tensor.matmul(
                            ps[:], fw1s[:, k, 128 * f : 128 * (f + 1)],
                            h2T[:, k, :],
                            start=(k == 0), stop=(k == F2 // 128 - 1),
                        )
                    nc.scalar.activation(
                        h3T[:, f, :], ps[:], ACT.Relu, bias=fb1t[:, f : f + 1]
                    )
                h4T = cls.tile([128, F4 // 128, R], dt.bfloat16)
                for f in range(F4 // 128):
                    ps = pp_g.tile([128, R], dt.float32, tag="gc", name=f"c2_{f}")
                    for k in range(F3 // 128):
                        nc.tensor.matmul(
                            ps[:], fw2s[:, k, 128 * f : 128 * (f + 1)],
                            h3T[:, k, :],
                            start=(k == 0), stop=(k == F3 // 128 - 1),
                        )
                    nc.scalar.activation(
                        h4T[:, f, :], ps[:], ACT.Relu, bias=fb2t[:, f : f + 1]
                    )
                pl = pp_g.tile([K, R], dt.float32, tag="gc", name="lgp")
                for k in range(F4 // 128):
                    nc.tensor.matmul(
                        pl[:], fw3s[:, k, :], h4T[:, k, :],
                        start=(k == 0), stop=(k == F4 // 128 - 1),
                    )
                lgt = cls.tile([K, R], dt.float32)
                nc.scalar.activation(lgt[:], pl[:], ACT.Identity, bias=fb3[:])

                # batched log-softmax + NLL over all 4 row-tiles.
                # logits are O(1) so the max-subtraction is skipped.
                pacc = pp_sm.tile([1, 1], dt.float32, tag="acc", bufs=1,
                                  name="lacc")
                plg = pp_t.tile([128, 4 * K], dt.float32, tag="tp", name="plg")
                for t in range(RT):
                    nc.tensor.transpose(
                        plg[:, K * t : K * (t + 1)],
                        lgt[:, 128 * t : 128 * (t + 1)], eye_f[0:K, 0:K]
                    )
                lgr = cls.tile([128, RT, K], dt.float32)
                nc.vector.tensor_copy(
                    lgr[:].rearrange("p t j -> p (t j)"), plg[:]
                )
                ex = cls.tile([128, RT, K], dt.float32)
                nc.scalar.activation(
                    ex[:].rearrange("p t j -> p (t j)"),
                    lgr[:].rearrange("p t j -> p (t j)"), ACT.Exp
                )
                sumex = cls.tile([128, RT, 1], dt.float32)
                nc.vector.reduce_sum(sumex[:], ex[:], axis=AX)
                lse = cls.tile([128, RT], dt.float32)
                nc.scalar.activation(
                    lse[:], sumex[:].rearrange("p t one -> p (t one)"), ACT.Ln
                )
                selm = cls.tile([128, RT, K], dt.float32)
                nc.vector.tensor_tensor(
                    selm[:].rearrange("p t j -> p (t j)"),
                    lgr[:].rearrange("p t j -> p (t j)"),
                    ohc[:].rearrange("p t j -> p (t j)"), OP.mult
                )
                selv = cls.tile([128, RT, 1], dt.float32)
                nc.vector.reduce_sum(selv[:], selm[:], axis=AX)
                nll = cls.tile([128, RT], dt.float32)
                nc.vector.tensor_tensor(
                    nll[:], lse[:], selv[:].rearrange("p t one -> p (t one)"),
                    OP.subtract
                )
                nlls = cls.tile([128, 1], dt.float32)
                nc.vector.reduce_sum(nlls[:], nll[:], axis=AX)
                nc.tensor.matmul(pacc[:], ones_c[:], nlls[:], start=True,
                                 stop=True)
                lsum = cls.tile([1, 1], dt.float32)
                nc.vector.tensor_copy(lsum[:], pacc[:])
                nc.sync.dma_start(
                    ls_in[:].rearrange("(p one) -> p one", one=1), lsum[:]
                )
                cc("AllGather", OP.bypass, ls_in, ls_out)
                lf = cls.tile([1, NC], dt.float32)
                nc.gpsimd.dma_start(
                    lf[:], ls_out[:].rearrange("(one k) -> one k", one=1)
                )
                lr = cls.tile([1, 1], dt.float32)
                nc.vector.reduce_sum(lr[:], lf[:], axis=AX)
                nc.vector.tensor_scalar_mul(lr[:], lr[:], 1.0 / N)
                nc.sync.dma_start(loss_d[:], lr[:])

    nc.finalize()
    return nc


_NC_CACHE = None


def kernel(x1, x2, label1, label2, W1, b1, W2, b2,
           fw1, fb1, fw2, fb2, fw3, fb3):
    global _NC_CACHE
    from concourse.bass_utils import run_bass_kernel_spmd

    x1 = np.asarray(x1, np.float32)
    x2 = np.asarray(x2, np.float32)
    label = np.concatenate([np.asarray(label1), np.asarray(label2)]).astype(np.int64)

    oh = np.zeros((N, K), np.float32)
    oh[np.arange(N), label] = 1.0
    su1 = np.maximum(oh[:2048].sum(0), 1.0)
    su2 = np.maximum(oh[2048:].sum(0), 1.0)
    ohdiv = np.concatenate([oh[:2048] / su1, oh[2048:] / su2], 0).astype(np.float32)

    # interleaved global order: rank r owns x1[256r:256r+256] ++ x2[256r:...]
    perm = np.concatenate(
        [np.r_[256 * r : 256 * (r + 1), 2048 + 256 * r : 2048 + 256 * (r + 1)]
         for r in range(NC)]
    )
    oh_g = np.ascontiguousarray(oh[perm])

    bf = ml_dtypes.bfloat16
    w1b = np.asarray(W1, np.float32).astype(bf)
    w2b = np.asarray(W2, np.float32).astype(bf)
    fw1b = np.asarray(fw1, np.float32).astype(bf)
    fw2b = np.asarray(fw2, np.float32).astype(bf)
    fw3b = np.asarray(fw3, np.float32).astype(bf)
    b1t = np.ascontiguousarray(np.asarray(b1, np.float32).reshape(F1 // 128, 128).T)
    b2t = np.ascontiguousarray(np.asarray(b2, np.float32).reshape(F2 // 128, 128).T)
    fb1t = np.ascontiguousarray(np.asarray(fb1, np.float32).reshape(F3 // 128, 128).T)
    fb2t = np.ascontiguousarray(np.asarray(fb2, np.float32).reshape(F4 // 128, 128).T)
    fb3c = np.asarray(fb3, np.float32).reshape(K, 1)

    if _NC_CACHE is None:
        _NC_CACHE = _build()
    nc = _NC_CACHE

    x = np.concatenate([x1, x2], 0)
    in_maps = []
    for c in range(NC):
        rows = perm[R * c : R * (c + 1)]
        in_maps.append({
            "xb": np.ascontiguousarray(x[rows]),
            "ohc": np.ascontiguousarray(oh[rows]),
            "ohdiv": np.ascontiguousarray(ohdiv[rows]),
            "ohall": oh_g,
            "w1b": w1b, "w2b": w2b, "fw1b": fw1b, "fw2b": fw2b, "fw3b": fw3b,
            "b1t": b1t, "b2t": b2t, "fb1t": fb1t, "fb2t": fb2t, "fb3c": fb3c,
        })

    res = run_bass_kernel_spmd(nc, in_maps, list(range(NC)))
    return np.asarray(res.results[0]["loss"], np.float32).reshape(())
